# revision 6
# baseline (speedup 1.0000x reference)
"""Variant B: n-split waves. See kernel.py docstring for the problem.

Waves cover an n-half (1024 cols, 2 PSUM banks) instead of the full N, so
the early phase only needs half the B stream; the second n-half runs
entirely from SBUF caches with zero input dependencies. The first two
waves are 2-m-tile superwaves (4 banks) so the PE consumes B rows at
roughly their arrival rate.
"""

import numpy as np

import concourse.bass as bass
import concourse.mybir as mybir
import concourse.tile as tile
from concourse import bacc
from concourse.bass_utils import run_bass_kernel_spmd

M_FULL, K, N_FULL = 4096, 2048, 8192
RI, CJ = 2, 4
M, N = M_FULL // RI, N_FULL // CJ
P = 128
KT = K // P
MT = M // P
NG = N // 1024                     # 2 n-groups of 1024
C_MAGIC = 12582912.0
CLIP = 127.0
NCORES = RI * CJ

F32 = mybir.dt.float32
BF16 = mybir.dt.bfloat16
AF = mybir.ActivationFunctionType

STN_BUFS = 6   # [P,1024] f32 input staging (rhs half k-rows)
STM_BUFS = 10  # [P,512] f32 input staging (lhsT chunks)
OST_BUFS = 6   # [P,512] f32 output staging
SYNC_OUT_WAVES = 4  # trailing waves whose outputs use the idle sync queue


def _build_nc(s_l, s_r, d_q):
    nc = bacc.Bacc("TRN2", target_bir_lowering=False, debug=False,
                   num_devices=NCORES)
    lhsT = nc.dram_tensor("lhsT", [K, M], F32, kind="ExternalInput")
    rhs = nc.dram_tensor("rhs", [K, N], F32, kind="ExternalInput")
    out = nc.dram_tensor("out", [M, N], F32, kind="ExternalOutput")

    with tile.TileContext(nc) as tc:
        _emit(nc, tc, lhsT, rhs, out, s_l, s_r, d_q)
    nc.compile()
    return nc


def _emit(nc, tc, lhsT, rhs, out, s_l, s_r, d_q):
    from contextlib import ExitStack
    ctx = ExitStack()
    with ctx:
        pstn = ctx.enter_context(tc.tile_pool(name="stn", bufs=STN_BUFS))
        pstm = ctx.enter_context(tc.tile_pool(name="stm", bufs=STM_BUFS))
        pcache = ctx.enter_context(tc.tile_pool(name="cache", bufs=1))
        ppsum = ctx.enter_context(tc.tile_pool(name="psum", bufs=8,
                                               space="PSUM"))
        post = ctx.enter_context(tc.tile_pool(name="ost", bufs=OST_BUFS))
        pconst = ctx.enter_context(tc.tile_pool(name="const", bufs=1))

        cb = pconst.tile([P, 1], F32, tag="cb")
        nc.vector.memset(cb[:], C_MAGIC)

        # persistent bf16 caches: qn[kt][g] = n-half g of rhs k-row kt;
        # qm[kt][c] = 512-col chunk of lhsT (chunk c feeds m-tiles 4c..4c+3)
        qn = [[pcache.tile([P, 1024], BF16, tag=f"qn{kt}_{g}",
                           name=f"qn{kt}_{g}") for g in range(NG)]
              for kt in range(KT)]
        qm = [[pcache.tile([P, 512], BF16, tag=f"qm{kt}_{c}",
                           name=f"qm{kt}_{c}")
               for c in range(1, 4)] for kt in range(KT)]
        qm0 = [[pcache.tile([P, 256], BF16, tag=f"qm0{kt}_{h}",
                            name=f"qm0{kt}_{h}")
                for h in range(2)] for kt in range(KT)]

        def quant_n(kt, g, chunks=1):
            st = pstn.tile([P, 1024], F32, tag="stn")
            w = 1024 // chunks
            for c in range(chunks):
                cs = slice(c * w, (c + 1) * w)
                gs = slice(g * 1024 + c * w, g * 1024 + (c + 1) * w)
                nc.sync.dma_start(st[:, cs], rhs[kt * P:(kt + 1) * P, gs])
                nc.scalar.activation(st[:, cs], st[:, cs], AF.Identity,
                                     bias=cb[:], scale=float(s_r))
                nc.vector.tensor_scalar_add(qn[kt][g][:, cs], st[:, cs],
                                            -C_MAGIC)

        def quant_m(kt, c):
            st = pstm.tile([P, 512], F32, tag="stm")
            nc.sync.dma_start(st[:], lhsT[kt * P:(kt + 1) * P,
                                          c * 512:(c + 1) * 512])
            nc.scalar.activation(st[:], st[:], AF.Identity, bias=cb[:],
                                 scale=float(s_l))
            nc.vector.tensor_scalar_add(qm[kt][c - 1][:], st[:], -C_MAGIC)

        def quant_m0(kt, h):
            st = pstm.tile([P, 512], F32, tag="stm")
            s2 = st[:, :256]
            nc.sync.dma_start(s2, lhsT[kt * P:(kt + 1) * P,
                                       h * 256:(h + 1) * 256])
            nc.scalar.activation(s2, s2, AF.Identity, bias=cb[:],
                                 scale=float(s_l))
            nc.vector.tensor_scalar_add(qm0[kt][h][:], s2, -C_MAGIC)

        def w_ap(kt, mt):
            if mt < 4:
                return qm0[kt][mt // 2][:, (mt % 2) * 128:(mt % 2 + 1) * 128]
            return qm[kt][mt // 4 - 1][:, (mt % 4) * 128:(mt % 4 + 1) * 128]

        def wave(mts, g, sync_out=False):
            psums = {(mt, nb): ppsum.tile([P, 512], F32, tag="ps",
                                          name=f"ps{mt}_{g}_{nb}")
                     for mt in mts for nb in range(2)}
            for kt in range(KT):
                for mt in mts:
                    w = w_ap(kt, mt)
                    for nb in range(2):
                        nc.tensor.matmul(
                            psums[mt, nb][:], w,
                            qn[kt][g][:, nb * 512:(nb + 1) * 512],
                            start=(kt == 0), stop=(kt == KT - 1))
            for mt in mts:
                m0 = mt * P
                for nb in range(2):
                    o = post.tile([P, 512], F32, tag="ost")
                    nc.scalar.activation(o[:], psums[mt, nb][:], AF.Copy,
                                         scale=float(d_q))
                    n0 = g * 1024 + nb * 512
                    eng = nc.sync if sync_out else nc.gpsimd
                    eng.dma_start(out[m0:m0 + P, n0:n0 + 512], o[:])

        # ---- emission schedule ----
        for kt in range(KT):
            quant_n(kt, 0, chunks=2 if kt < 2 else 1)
            quant_m0(kt, 0)
        wave([0, 1], 0)                      # superwave: 4 banks
        for kt in range(KT):
            quant_m0(kt, 1)
        wave([2, 3], 0)                      # superwave: 4 banks
        for kt in range(KT):
            quant_m(kt, 1)
        for mt in range(4, 8):
            wave([mt], 0)
        for kt in range(KT):
            quant_m(kt, 2)
        for mt in range(8, 12):
            wave([mt], 0)
            for kt in range(4 * (mt - 8), 4 * (mt - 7)):
                quant_n(kt, 1)
        for kt in range(KT):
            quant_m(kt, 3)
        for mt in range(12, 16):
            wave([mt], 0)
        # n-half 1: fully cached, zero input deps
        for mt in range(16):
            wave([mt], 1,
                 sync_out=(mt >= 16 - SYNC_OUT_WAVES))


_NC_CACHE = {}


def _get_nc(s_l, s_r, d_q):
    key = (float(s_l), float(s_r), float(d_q))
    if key not in _NC_CACHE:
        _NC_CACHE[key] = _build_nc(*key)
    return _NC_CACHE[key]


def _host_scales(lhs, rhs):
    ml = np.maximum(np.abs(lhs).max(), np.float32(1e-6))
    mr = np.maximum(np.abs(rhs).max(), np.float32(1e-6))
    s_l = np.float32(CLIP) / ml
    s_r = np.float32(CLIP) / mr
    d_q = (np.float32(1.0) / s_l) * (np.float32(1.0) / s_r)
    return s_l, s_r, d_q


LAST_RESULT = None


def kernel(lhs, rhs, _trace=False, _trace_cores=None):
    global LAST_RESULT
    lhs = np.ascontiguousarray(np.asarray(lhs, dtype=np.float32))
    rhs = np.ascontiguousarray(np.asarray(rhs, dtype=np.float32))
    assert lhs.shape == (M_FULL, K) and rhs.shape == (K, N_FULL)

    lhsT = np.ascontiguousarray(lhs.T)
    s_l, s_r, d_q = _host_scales(lhs, rhs)

    in_maps = []
    for i in range(RI):
        lT = np.ascontiguousarray(lhsT[:, i * M:(i + 1) * M])
        for j in range(CJ):
            r = np.ascontiguousarray(rhs[:, j * N:(j + 1) * N])
            in_maps.append({"lhsT": lT, "rhs": r})

    nc = _get_nc(s_l, s_r, d_q)
    res = run_bass_kernel_spmd(
        nc, in_maps, core_ids=list(range(NCORES)),
        trace=_trace,
        **({"trace_cores": _trace_cores} if _trace_cores else {}))
    LAST_RESULT = res

    full = np.empty((M_FULL, N_FULL), dtype=np.float32)
    for i in range(RI):
        for j in range(CJ):
            full[i * M:(i + 1) * M, j * N:(j + 1) * N] = \
                res.results[i * CJ + j]["out"]
    return full
